# revision 1
# baseline (speedup 1.0000x reference)
"""AI4DEM contact-force kernel for 8 TRN2 NeuronCores.

Data-parallel over particles with z-sharded, pre-windowed voxel grids:
  - Host: cell binning, mixer update, scatter into windowed grid
    E[z10][x256][y'260][64] (row = 5-cell x-window x 9 fields), shard by
    z-slab, group by (z-slice, x-third) so gather indices fit int16.
  - Device: per 640-particle subbatch, 5 dma_gathers (one per dz) pull
    320-f32 elements (5 dy-rows = dy5 x dx5 x f9 block); the 125-offset
    contact physics runs as [128, 625] DVE/ACT/GpSimd sweeps; per-offset
    contributions reduce on-chip to (6, NP).
  - Host: inverse permutation back to (6, N).
"""

import os
import numpy as np

DZG, DYG, DXG = 48, 256, 256
N = 400000
D = 0.00054
KN = 200.0
REST = 0.3
_alpha = -np.log(REST) / np.pi
_gamma = _alpha / np.sqrt(_alpha**2 + 1.0)
MASS = 4.0 / 3.0 * 3.1415 * D**3 * 2500.0
ETA = float(2.0 * _gamma * np.sqrt(KN * MASS / 2.0))
MU = 0.8
DT = 1e-05
EPS = 1e-04
MIXER_W = 1.0
MIXER_R = 50

NCORES = 8
SLAB = DZG // NCORES
XT_BASES = (0, 86, 172)
XT_WIDTHS = (86, 86, 84)
NGRP = SLAB * 3               # 18 groups (czrel, xthird)
GRP = 3072
SB = 384
NSB_PER_GRP = GRP // SB       # 5
CC = SB // 128                # 5
NPAD = NGRP * GRP             # 57600
NCOL = NPAD // 128            # 450
ELEM = 320
STEP = 64
YH = DYG + 4                  # 260
EROWS_PER_Z = DXG * YH

LAST_RESULT = None


def _mixer_update(x, y, z, vx, vy, vz, group3):
    g3 = group3 != 0
    f = np.float32
    vx = np.where(g3, (-MIXER_W * (y - f((MIXER_R - 1) * D))).astype(f), vx)
    vy = np.where(g3, (MIXER_W * (x - f((MIXER_R - 1) * D))).astype(f), vy)
    vz = np.where(g3, f(0.0), vz)
    x = np.where(g3, (x + f(DT) * vx).astype(f), x)
    y = np.where(g3, (y + f(DT) * vy).astype(f), y)
    z = np.where(g3, (z + f(DT) * vz).astype(f), z)
    return x, y, z, vx, vy, vz


_GRAPH = None


def _build_graph(groups):
    import concourse.bacc as bacc
    import concourse.mybir as mybir
    import concourse.tile as tile
    from concourse import library_config

    f32 = mybir.dt.float32
    Alu = mybir.AluOpType
    Act = mybir.ActivationFunctionType

    nc = bacc.Bacc("TRN2", target_bir_lowering=False, debug=False,
                   enable_asserts=False, num_devices=NCORES)
    E = nc.dram_tensor("E", [10 * EROWS_PER_Z * STEP], f32, kind="ExternalInput")
    idx_d = nc.dram_tensor("idx", [128, NGRP * NSB_PER_GRP * (SB // 16)],
                           mybir.dt.int16, kind="ExternalInput")
    pf_d = nc.dram_tensor("pf", [9, 128, NCOL], f32, kind="ExternalInput")
    out_d = nc.dram_tensor("out", [6, 128, NCOL], f32, kind="ExternalOutput")

    def apo(ap, dims, extra_off=0):
        v = ap.copy()
        v.ap = type(v.ap)([[ap.ap[0][0], ap.ap[0][1]]] + list(dims))
        v.offset = v.offset + extra_off
        return v

    EPS2 = float(EPS * EPS)
    FOURD2 = float(4.0 * D * D)
    FREE = CC * 125

    relu_bias = float(KN * 2.0 * D)
    _cb = nc.alloc_sbuf_tensor("const-relu-bias", [128, 1], f32)
    nc.gpsimd.memset(_cb.ap(), relu_bias)
    nc.const_aps.aps[(f32, relu_bias)] = _cb.ap()

    with tile.TileContext(nc) as tc:
        with (
            tc.tile_pool(name="resident", bufs=1) as rp,
            tc.tile_pool(name="nbp", bufs=2) as nbp,
            tc.tile_pool(name="tmp", bufs=2) as tp,
            tc.tile_pool(name="scr", bufs=2) as scp,
        ):
            nc.gpsimd.load_library(library_config.mlp)
            idx_t = rp.tile([128, NGRP * NSB_PER_GRP * (SB // 16)], mybir.dt.int16)
            nc.sync.dma_start(idx_t[:], idx_d[:])
            pft = []
            for f in range(9):
                t = rp.tile([128, NCOL], f32, tag=f"pf{f}")
                nc.sync.dma_start(t[:], pf_d[f])
                pft.append(t)
            oacc = [rp.tile([128, NCOL], f32, tag=f"oacc{c}", name=f"oacc{c}") for c in range(6)]

            for g in groups:
                czrel, xt = divmod(g, 3)
                for s in range(NSB_PER_GRP):
                    sbi = g * NSB_PER_GRP + s
                    c0 = sbi * CC
                    nb = nbp.tile([128, 5 * CC * ELEM], f32, tag="nb")
                    for dz in range(5):
                        zsl = czrel + dz
                        base = (zsl * DXG + XT_BASES[xt]) * YH * STEP
                        nrows = XT_WIDTHS[xt] * YH - 4
                        src = E[:].copy()
                        src.ap = type(src.ap)([[STEP, nrows], [1, ELEM]])
                        src.offset = src.offset + base
                        nc.gpsimd.dma_gather(
                            out_ap=nb[:, dz * CC * ELEM:(dz + 1) * CC * ELEM]
                                .rearrange("p (c e) -> p c e", e=ELEM),
                            in_ap=src,
                            idxs_ap=idx_t[:, sbi * (SB // 16):(sbi + 1) * (SB // 16)],
                            num_idxs=SB, num_idxs_reg=SB,
                            elem_size=ELEM, elem_step=STEP,
                        )

                    # nb free layout: [(dz,c)25 @320][dy5 @64][dx5 @9] + f
                    def NB(f):
                        return apo(nb[:], [[ELEM, 5 * CC], [STEP, 5], [9, 5]], f)

                    def CB(f):
                        return apo(pft[f][:], [[0, 5], [1, CC], [0, 25]], c0)

                    def T(name):
                        return tp.tile([128, FREE], f32, tag=name, name=name)

                    V, A, G_ = nc.vector, nc.scalar, nc.gpsimd

                    dxp, dyp, dzp = T("dxp"), T("dyp"), T("dzp")
                    dvx, dvy, dvz = T("dvx"), T("dvy"), T("dvz")
                    wxs, wys, wzs = T("wxs"), T("wys"), T("wzs")
                    V.tensor_tensor(dxp[:], CB(0), NB(0), Alu.subtract)
                    V.tensor_tensor(dyp[:], CB(1), NB(1), Alu.subtract)
                    V.tensor_tensor(dzp[:], CB(2), NB(2), Alu.subtract)
                    V.tensor_tensor(dvx[:], CB(3), NB(3), Alu.subtract)
                    V.tensor_tensor(dvy[:], CB(4), NB(4), Alu.subtract)
                    V.tensor_tensor(dvz[:], CB(5), NB(5), Alu.subtract)
                    G_.tensor_tensor(wxs[:], CB(6), NB(6), Alu.add)
                    G_.tensor_tensor(wys[:], CB(7), NB(7), Alu.add)
                    G_.tensor_tensor(wzs[:], CB(8), NB(8), Alu.add)

                    t0, t1, t2 = T("t0"), T("t1"), T("t2")
                    inv, mask = T("inv"), T("mask")
                    fnsum, fmag, b_ = T("fnsum"), T("fmag"), T("b_")

                    A.activation(t0[:], dxp[:], Act.Square)
                    A.activation(t1[:], dyp[:], Act.Square)
                    A.activation(t2[:], dzp[:], Act.Square)
                    V.tensor_tensor(t0[:], t0[:], t1[:], Alu.add)
                    V.tensor_tensor(t0[:], t0[:], t2[:], Alu.add)
                    V.tensor_scalar(t0[:], t0[:], EPS2, None, Alu.max)  # d2c
                    V.tensor_scalar(mask[:], t0[:], FOURD2, None, Alu.is_lt)
                    A.activation(t1[:], t0[:], Act.Sqrt)                # dist
                    V.reciprocal_approx_accurate(inv[:], t1[:], t0[:])
                    A.activation(t2[:], t1[:], Act.Relu,
                                 bias=relu_bias, scale=float(-KN))  # fnc

                    V.tensor_tensor(t1[:], dvx[:], dxp[:], Alu.mult)
                    V.tensor_tensor(t0[:], dvy[:], dyp[:], Alu.mult)
                    V.tensor_tensor(t1[:], t1[:], t0[:], Alu.add)
                    V.tensor_tensor(t0[:], dvz[:], dzp[:], Alu.mult)
                    V.tensor_tensor(t1[:], t1[:], t0[:], Alu.add)       # Vn*dd
                    V.tensor_tensor(t1[:], t1[:], inv[:], Alu.mult)     # Vn
                    G_.tensor_tensor(t0[:], t1[:], mask[:], Alu.mult)   # fnd0
                    V.tensor_tensor(t1[:], t1[:], inv[:], Alu.mult)     # a_ (in t1)
                    V.scalar_tensor_tensor(fnsum[:], t0[:], float(-ETA), t2[:],
                                           Alu.mult, Alu.add)
                    A.activation(t0[:], t0[:], Act.Abs)
                    V.scalar_tensor_tensor(fmag[:], t0[:], float(ETA), t2[:],
                                           Alu.mult, Alu.add)
                    A.activation(b_[:], inv[:], Act.Copy, scale=float(D))

                    armx, army, armz = T("armx"), T("army"), T("armz")
                    G_.tensor_tensor(armx[:], b_[:], dxp[:], Alu.mult)
                    G_.tensor_tensor(army[:], b_[:], dyp[:], Alu.mult)
                    G_.tensor_tensor(armz[:], b_[:], dzp[:], Alu.mult)

                    vt = [T("vtx"), T("vty"), T("vtz")]
                    arms = (armx, army, armz)
                    dps = (dxp, dyp, dzp)
                    dvs = (dvx, dvy, dvz)
                    ws = (wxs, wys, wzs)
                    for i in range(3):
                        j, k = (i + 1) % 3, (i + 2) % 3
                        G_.tensor_tensor(t0[:], ws[j][:], arms[k][:], Alu.mult)
                        G_.tensor_tensor(t2[:], ws[k][:], arms[j][:], Alu.mult)
                        V.tensor_tensor(t0[:], t0[:], t2[:], Alu.subtract)
                        V.tensor_tensor(t0[:], dvs[i][:], t0[:], Alu.add)
                        V.tensor_tensor(t2[:], t1[:], dps[i][:], Alu.mult)
                        V.tensor_tensor(vt[i][:], t0[:], t2[:], Alu.subtract)

                    A.activation(t0[:], vt[0][:], Act.Square)
                    A.activation(t1[:], vt[1][:], Act.Square)
                    A.activation(t2[:], vt[2][:], Act.Square)
                    V.tensor_tensor(t0[:], t0[:], t1[:], Alu.add)
                    V.tensor_tensor(t0[:], t0[:], t2[:], Alu.add)
                    V.tensor_scalar(t0[:], t0[:], EPS2, None, Alu.max)
                    A.activation(t2[:], t0[:], Act.Sqrt)                # vt
                    V.reciprocal_approx_accurate(t1[:], t2[:], t0[:])   # 1/vt
                    ft0 = T("ft0")
                    V.scalar_tensor_tensor(ft0[:], fmag[:], float(-MU), t1[:],
                                           Alu.mult, Alu.mult)
                    ff = [T("ffx"), T("ffy"), T("ffz")]
                    for i in range(3):
                        V.tensor_tensor(ff[i][:], ft0[:], vt[i][:], Alu.mult)
                    V.tensor_tensor(t0[:], fnsum[:], inv[:], Alu.mult)  # c2

                    SCRD = [[25, 5], [125, CC], [1, 25]]
                    scr = [scp.tile([128, FREE], f32, tag=f"scr{c}", name=f"scr{c}") for c in range(6)]
                    for i in range(3):
                        V.tensor_tensor(t1[:], t0[:], dps[i][:], Alu.mult)
                        V.tensor_tensor(apo(scr[i][:], SCRD), t1[:], ff[i][:],
                                        Alu.add)
                    for i in range(3):
                        j, k = (i + 1) % 3, (i + 2) % 3
                        G_.tensor_tensor(t1[:], arms[j][:], ff[k][:], Alu.mult)
                        G_.tensor_tensor(t2[:], arms[k][:], ff[j][:], Alu.mult)
                        V.tensor_tensor(apo(scr[3 + i][:], SCRD), t1[:], t2[:],
                                        Alu.subtract)
                    for c in range(6):
                        V.tensor_reduce(
                            oacc[c][:, c0:c0 + CC],
                            scr[c][:].rearrange("p (c r) -> p c r", r=125),
                            mybir.AxisListType.X, Alu.add)

            for c in range(6):
                nc.sync.dma_start(out_d[c], oacc[c][:])

    nc.compile()
    return nc


def kernel(x, y, z, vx, vy, vz, wx, wy, wz, group3, _groups=None):
    global _GRAPH, LAST_RESULT
    from concourse.bass_utils import run_bass_kernel_spmd

    x = np.asarray(x, np.float32); y = np.asarray(y, np.float32)
    z = np.asarray(z, np.float32)
    vx = np.asarray(vx, np.float32); vy = np.asarray(vy, np.float32)
    vz = np.asarray(vz, np.float32)
    wx = np.asarray(wx, np.float32); wy = np.asarray(wy, np.float32)
    wz = np.asarray(wz, np.float32)
    group3 = np.asarray(group3, np.int32)

    cx = np.round(x / np.float32(D)).astype(np.int64)
    cy = np.round(y / np.float32(D)).astype(np.int64)
    cz = np.round(z / np.float32(D)).astype(np.int64)
    x2, y2, z2, vx2, vy2, vz2 = _mixer_update(x, y, z, vx, vy, vz, group3)
    fields = np.stack([x2, y2, z2, vx2, vy2, vz2, wx, wy, wz], axis=1)

    F = np.zeros((DZG, DYG, DXG, 9), np.float32)
    F[cz, cy, cx] = fields
    Ft = np.ascontiguousarray(F.transpose(0, 2, 1, 3))               # [z][x][y][9]
    Fp = np.concatenate([Ft[:, :, -2:], Ft, Ft[:, :, :2]], axis=2)   # y-halo
    E_g = np.zeros((DZG, DXG, YH, STEP), np.float32)
    for dx in range(5):
        E_g[..., dx * 9:(dx + 1) * 9] = np.roll(Fp, 2 - dx, axis=1)

    xt = cx // 86
    core = cz // SLAB
    czrel = cz % SLAB
    key = (core * NGRP + czrel * 3 + xt).astype(np.int64)
    order = np.argsort(key, kind="stable")
    sizes = np.bincount(key, minlength=NCORES * NGRP)
    assert sizes.max() <= GRP, f"group overflow: {sizes.max()}"

    groups = list(range(NGRP)) if _groups is None else list(_groups)
    if _GRAPH is None:
        _GRAPH = _build_graph(groups)
    nc = _GRAPH

    in_maps = []
    slot_orig = np.full((NCORES, NPAD), -1, np.int64)
    starts = np.concatenate([[0], np.cumsum(sizes)])
    for i in range(NCORES):
        zsl = [(i * SLAB - 2 + k) % DZG for k in range(10)]
        E_i = np.ascontiguousarray(E_g[zsl]).reshape(-1)
        idx_i = np.zeros((128, NGRP * NSB_PER_GRP * (SB // 16)), np.int16)
        pf_i = np.zeros((9, 128, NCOL), np.float32)
        for g in range(NGRP):
            k = i * NGRP + g
            members = order[starts[k]:starts[k + 1]]
            n = len(members)
            if n == 0:
                continue
            jj = np.arange(n)
            p = jj % 128
            col = g * (NSB_PER_GRP * CC) + (jj // 128)
            slot_orig[i, g * GRP + jj] = members
            pf_i[:, p, col] = fields[members].T
            xtg = g % 3
            iv = ((cx[members] - XT_BASES[xtg]) * YH + cy[members]).astype(np.int16)
            t_in_sb = jj % SB
            cidx = (g * NSB_PER_GRP + jj // SB) * (SB // 16) + t_in_sb // 16
            ridx = t_in_sb % 16
            for rgrp in range(8):
                idx_i[rgrp * 16 + ridx, cidx] = iv
        in_maps.append({"E": E_i, "idx": idx_i, "pf": pf_i})

    res = run_bass_kernel_spmd(nc, in_maps, core_ids=list(range(NCORES)),
                               trace=bool(os.environ.get("K_TRACE")))
    LAST_RESULT = res

    out = np.zeros((6, N), np.float32)
    jj = np.arange(NPAD)
    pall = jj % 128
    call = (jj // GRP) * (NSB_PER_GRP * CC) + (jj % GRP) // 128
    for i in range(NCORES):
        o = res.results[i]["out"]
        so = slot_orig[i]
        valid = so >= 0
        out[:, so[valid]] = o[:, pall[valid], call[valid]]
    return out

